# revision 12
# baseline (speedup 1.0000x reference)
"""COLoRALinear fused kernel for 8 trn2 NeuronCores (Bass/Tile).

Problem: out = x@W.T + b + cw*2*(x@sA.T)@sB.T + (1-cw)*2*sum_t r[b,t]*(x@tA[t].T)@tB[t].T
with routing r = softmax(mean_s(x) @ emb.T), cw = sigmoid(collab_weight).

v3 sharding: core i -> batch element p=i//2, token half h=i%2 (1024
tokens), FULL dout (4096). vs the dout-split v2 this halves the
duplicated phase A (A_cat @ x^T runs over 1024 tokens instead of 2048)
and moves the routing token-mean off the PE:
  - x mean partials: per-k-tile free-dim reduces of the OTHER token
    half on GpSimd (mid-rep, fully slack) and of MY half on DVE at the
    rep boundary; logits = emb @ mean via a chain of 64 N=1 matmuls.
  - phase A: 64 MMs (kt-outer, 2 chunks of 512) -> hid[72, 1024].
  - main: 32 d-tiles x 32 kt x 2 chunks N=512 base MMs + 2 down-proj
    MMs per d-tile (lhsT = routing-scaled B_cat, rhs = hid).
  - W is now streamed in FULL per core (32 d-tiles, host-tiled
    [128, 8KB] bf16 contiguous DMAs) - HBM has the headroom.
PE stream/rep ~= 2048*216 + 64*216 + 64*216 + 64*52 ns ~= 473 us vs
v2's 484 us. All inputs bf16 (host cast); output [dout, 1024] bf16.
x rings: my-half 38-deep, other-half 34-deep [128,1024] tiles so the
next rep's preload hides under the current rep's tail.
"""
import numpy as np
import ml_dtypes
from contextlib import ExitStack

import concourse.bass as bass
import concourse.tile as tile
from concourse import mybir
from concourse.bass_utils import run_bass_kernel_spmd
from concourse.vector_clock import ScopedClock

B, S, DIN, DOUT, R, T = 4, 2048, 4096, 4096, 8, 8
SCALING = 2.0
N_CORES = 8
P = 128
KT = DIN // P            # 32 k-tiles
S_CORE = S // 2          # tokens per core (half a batch element)
ND = DOUT // P           # 32 d-tiles (full dout)
NCH = 512                # psum chunk width (1 bank)
NC2 = S_CORE // NCH      # 2 chunks
A72 = 72                 # 8 shared + 64 task rows
HID = 73                 # 72 lora rows + ones(bias) row
XMY_BUFS = 44            # my-half x ring
XOT_BUFS = 16           # other-half x ring
F32 = mybir.dt.float32
BF16 = mybir.dt.bfloat16
BF16NP = ml_dtypes.bfloat16


class _DrainSplitTileContext(tile.TileContext):
    """Walrus in this container rejects a Drain carrying >1 sem wait (the
    CTRL_NO encoding has one TPB_EVENTS wait slot). Split the exit drain's
    waits across a chain of single-wait drains."""

    def _drain_and_barrier(self, tick_clock, wait_clock):
        drain_inst = self.nc.sync.drain()
        wait_clock.add_sem_waits(
            drain_inst.ins, ScopedClock({None: tick_clock.global_clock})
        )
        si = drain_inst.ins.sync_info
        if si is not None and len(si.on_wait) > 1:
            waits = list(si.on_wait)
            drain_inst.ins.sync_info = mybir.SyncInfo(
                on_wait=[waits[0]], on_update=list(si.on_update)
            )
            for w in waits[1:]:
                extra = self.nc.sync.drain()
                extra.ins.sync_info = mybir.SyncInfo(on_wait=[w], on_update=[])

        self.nc.all_engine_barrier()
        assert self.sems is not None
        popped = self.nc._tile_sem_poison_stack.pop()
        assert popped is self._sem_poison
        self.nc.clear_and_free_semaphores(list(self.sems.allocated().values()))
        self.nc.all_engine_barrier()


_wsplit_counter = [0]


def _split_multi_waits(nc):
    """Walrus here lowers DMA/CTRL instructions with a single TPB_EVENTS wait
    slot and rejects >1 sem wait. Hoist extra waits onto same-engine NoOps
    inserted immediately before the offending instruction (engine program
    order makes this semantics-preserving)."""
    for f in nc.m.functions:
        for blk in f.blocks:
            insts = blk.instructions
            out = []
            changed = False
            for inst in insts:
                si = inst.sync_info
                if si is not None and len(si.on_wait) > 1:
                    waits = list(si.on_wait)
                    for w in waits[:-1]:
                        _wsplit_counter[0] += 1
                        nop = mybir.InstNoOp(name=f"I-wsplit-{_wsplit_counter[0]}")
                        nop.engine = inst.engine
                        nop.sync_info = mybir.SyncInfo(on_wait=[w], on_update=[])
                        out.append(nop)
                    inst.sync_info = mybir.SyncInfo(
                        on_wait=[waits[-1]], on_update=list(si.on_update)
                    )
                    changed = True
                out.append(inst)
            if changed:
                blk.instructions = out


def build_nc(reps: int = 1):
    nc = bass.Bass(trn_type="TRN2", target_bir_lowering=False)
    xmy = nc.dram_tensor("xmy", [DIN, S_CORE], BF16, kind="ExternalInput").ap()
    xot = nc.dram_tensor("xot", [DIN, S_CORE], BF16, kind="ExternalInput").ap()
    wt = nc.dram_tensor("wt", [ND, P, KT * P], BF16, kind="ExternalInput").ap()
    a72 = nc.dram_tensor("a72", [P, KT, A72], BF16, kind="ExternalInput").ap()
    emb = nc.dram_tensor("emb", [P, KT, T], F32, kind="ExternalInput").ap()
    bcat = nc.dram_tensor("bcat", [HID, DOUT], BF16, kind="ExternalInput").ap()
    cw = nc.dram_tensor("cw", [1, 1], F32, kind="ExternalInput").ap()
    # output stored [dout, tok] bf16; host upconverts + transposes
    out = nc.dram_tensor("out", [DOUT, S_CORE], BF16, kind="ExternalOutput").ap()

    xmy_r = xmy.rearrange("(kt p) t -> p kt t", p=P)
    xot_r = xot.rearrange("(kt p) t -> p kt t", p=P)
    out_r = out.rearrange("(d p) t -> p d t", p=P)

    with _DrainSplitTileContext(nc) as tc, ExitStack() as ctx:
        xp = ctx.enter_context(tc.tile_pool(name="xp", bufs=1))
        wch_p = ctx.enter_context(tc.tile_pool(name="wch", bufs=3))
        cst_p = ctx.enter_context(tc.tile_pool(name="cst", bufs=2))
        small_p = ctx.enter_context(tc.tile_pool(name="small", bufs=2))
        pers_p = ctx.enter_context(tc.tile_pool(name="pers", bufs=1))
        evict_p = ctx.enter_context(tc.tile_pool(name="ev", bufs=3))
        ps_p = ctx.enter_context(tc.tile_pool(name="ps", bufs=2, space="PSUM"))

        # persistent: ones(bias) row of hid; hid/bbf rewritten per rep
        hid = pers_p.tile([HID, S_CORE], BF16)
        ones_row = pers_p.tile([1, S_CORE], BF16)
        nc.vector.memset(ones_row[:], 1.0)
        nc.sync.dma_start(out=hid[72:73, :], in_=ones_row[:])
        bbf = pers_p.tile([HID, DOUT], BF16)

        st = {}  # per-rep emitted state: consts, xot sums

        def emit_consts(r):
            s = {}
            s["a_bf"] = cst_p.tile([P, KT, A72], BF16, tag="abf", name=f"a_bf_{r}")
            nc.gpsimd.dma_start(out=s["a_bf"][:], in_=a72)
            s["embt"] = cst_p.tile([P, KT, T], F32, tag="embt", name=f"embt_{r}")
            nc.gpsimd.dma_start(out=s["embt"][:], in_=emb)
            s["bmat"] = cst_p.tile([HID, DOUT], BF16, tag="bmat", name=f"bmat_{r}")
            nc.gpsimd.dma_start(out=s["bmat"][:], in_=bcat)
            cwt = small_p.tile([1, 1], F32, tag="cwt", name=f"cwt_{r}")
            nc.scalar.dma_start(out=cwt[:], in_=cw)
            sig = small_p.tile([1, 1], F32, tag="sig", name=f"sig_{r}")
            nc.scalar.activation(
                out=sig[:], in_=cwt[:], func=mybir.ActivationFunctionType.Sigmoid
            )
            s["cw2"] = small_p.tile([1, 1], F32, tag="cw2", name=f"cw2_{r}")
            nc.vector.tensor_scalar_mul(s["cw2"][:], sig[:], SCALING)
            s["tsc"] = small_p.tile([1, 1], F32, tag="tsc", name=f"tsc_{r}")
            nc.vector.tensor_scalar(
                out=s["tsc"][:], in0=sig[:], scalar1=-SCALING, scalar2=SCALING,
                op0=mybir.AluOpType.mult, op1=mybir.AluOpType.add,
            )
            s["xmo"] = small_p.tile([P, KT], F32, tag="xmo", name=f"xmo_{r}")
            st[r] = s

        def emit_xot_piece(r, kts):
            # other-half x: DMA (scalar queue, BEFORE this iteration's out-dma)
            # then free-dim token-sum on DVE (emitted by caller AFTER evicts)
            s = st[r]
            tiles = []
            for kt in kts:
                xk = xp.tile([P, S_CORE], BF16, tag="xo", bufs=XOT_BUFS,
                             name=f"xo_{r}_{kt}")
                nc.gpsimd.dma_start(out=xk[:], in_=xot_r[:, kt, :])
                tiles.append((kt, xk))
            s.setdefault("xo_tiles", []).extend(tiles)
            return tiles

        def emit_xmo_reduce(r, tiles):
            s = st[r]
            for kt, xk in tiles:
                nc.vector.tensor_reduce(
                    out=s["xmo"][:, kt:kt + 1], in_=xk[:],
                    axis=mybir.AxisListType.X, op=mybir.AluOpType.add,
                )

        def emit_head(r):
            # my-half x ring (sync queue) + boundary token-sums (DVE)
            s = st[r]
            xs = []
            for kt in range(KT):
                xk = xp.tile([P, S_CORE], BF16, tag="xm", bufs=XMY_BUFS,
                             name=f"xm_{r}_{kt}")
                nc.sync.dma_start(out=xk[:], in_=xmy_r[:, kt, :])
                xs.append(xk)
            s["xs"] = xs
            xmm = small_p.tile([P, KT], F32, tag="xmm", name=f"xmm_{r}")
            for kt in range(KT):
                nc.vector.tensor_reduce(
                    out=xmm[:, kt:kt + 1], in_=xs[kt][:],
                    axis=mybir.AxisListType.X, op=mybir.AluOpType.add,
                )
            s["xmm"] = xmm
            # phase A: kt-outer over 2 chunks
            pa = [
                ps_p.tile([A72, NCH], F32, tag=f"ps{c}", name=f"pa{c}_{r}")
                for c in range(NC2)
            ]
            for kt in range(KT):
                for c in range(NC2):
                    nc.tensor.matmul(
                        pa[c][:], lhsT=s["a_bf"][:, kt, :],
                        rhs=xs[kt][:, c * NCH:(c + 1) * NCH],
                        start=(kt == 0), stop=(kt == KT - 1),
                    )
            for c in range(NC2):
                nc.vector.tensor_copy(
                    out=hid[0:72, c * NCH:(c + 1) * NCH], in_=pa[c][0:72, :]
                )

        def emit_rt_piece(r, rt, idx):
            # 3 of the 64 logit-accum matmuls (emb @ token-sums), slotted
            # between d0's dense base matmuls to keep HAM warm
            s = st[r]
            for i in range(idx, min(idx + 3, 2 * KT)):
                half, kt = divmod(i, KT)
                nc.tensor.matmul(
                    rt[:], lhsT=s["embt"][:, kt, :],
                    rhs=(s["xmm"] if half == 0 else s["xmo"])[:, kt:kt + 1],
                    start=(i == 0), stop=(i == 2 * KT - 1),
                )

        def emit_softmax(r, rt):
            s = st[r]
            rts = small_p.tile([T, 1], F32, tag="rts", name=f"rts_{r}")
            nc.vector.tensor_copy(out=rts[:], in_=rt[:])
            l_row = small_p.tile([1, T], F32, tag="l_row", name=f"l_row_{r}")
            nc.gpsimd.dma_start(out=l_row[:], in_=rts[:])  # partition->free
            e_row = small_p.tile([1, T], F32, tag="e_row", name=f"e_row_{r}")
            nc.scalar.activation(
                out=e_row[:], in_=l_row[:], func=mybir.ActivationFunctionType.Exp,
                scale=1.0 / S,
            )
            ssum = small_p.tile([1, 1], F32, tag="ssum", name=f"ssum_{r}")
            nc.vector.tensor_reduce(
                out=ssum[:], in_=e_row[:], axis=mybir.AxisListType.X,
                op=mybir.AluOpType.add,
            )
            rec = small_p.tile([1, 1], F32, tag="rec", name=f"rec_{r}")
            nc.vector.reciprocal(out=rec[:], in_=ssum[:])
            comb = small_p.tile([1, 1], F32, tag="comb", name=f"comb_{r}")
            nc.vector.tensor_tensor(
                out=comb[:], in0=rec[:], in1=s["tsc"][:], op=mybir.AluOpType.mult
            )
            ones8 = small_p.tile([1, T], F32, tag="ones8", name=f"ones8_{r}")
            nc.vector.memset(ones8[:], 1.0)
            svec_f = small_p.tile([1, HID], F32, tag="svec_f", name=f"svec_f_{r}")
            nc.vector.tensor_scalar(
                out=svec_f[0:1, 0:8], in0=ones8[:], scalar1=s["cw2"][:],
                scalar2=None, op0=mybir.AluOpType.mult,
            )
            for t in range(T):
                nc.vector.tensor_scalar(
                    out=svec_f[0:1, 8 + 8 * t:16 + 8 * t], in0=ones8[:],
                    scalar1=e_row[0:1, t:t + 1], scalar2=comb[:],
                    op0=mybir.AluOpType.mult, op1=mybir.AluOpType.mult,
                )
            nc.vector.memset(svec_f[0:1, 72:73], 1.0)
            svec = small_p.tile([HID, 1], F32, tag="svec", name=f"svec_{r}")
            nc.gpsimd.dma_start(out=svec[:], in_=svec_f[:])  # free->partition
            nc.vector.tensor_scalar(
                out=bbf[:], in0=s["bmat"][:], scalar1=svec[:], scalar2=None,
                op0=mybir.AluOpType.mult,
            )

        def emit_main(r):
            s = st[r]
            xs = s["xs"]
            nxt = r + 1 if r + 1 < reps else None
            for d in range(ND):
                # next rep's constants + other-half stream, pipelined in
                if nxt is not None and d == 1:
                    emit_consts(nxt)
                pend = None
                if nxt is not None and 2 <= d < 2 + KT // 2:
                    pend = emit_xot_piece(nxt, [2 * (d - 2), 2 * (d - 2) + 1])
                wch = wch_p.tile([P, KT * P], BF16, tag="wch", name=f"wch_{r}_{d}")
                nc.sync.dma_start(out=wch[:], in_=wt[d])
                pss = [
                    ps_p.tile([P, NCH], F32, tag=f"ps{c}", name=f"ps{c}_{r}_{d}")
                    for c in range(NC2)
                ]
                rt = None
                if d == 0:
                    rt = ps_p.tile([T, 1], F32, tag="rt", name=f"rt_{r}")
                for kt in range(KT):
                    for c in range(NC2):
                        nc.tensor.matmul(
                            pss[c][:], lhsT=wch[:, kt * P:(kt + 1) * P],
                            rhs=xs[kt][:, c * NCH:(c + 1) * NCH],
                            start=(kt == 0), stop=False,
                        )
                    if d == 0 and kt >= 4:
                        emit_rt_piece(r, rt, 3 * (kt - 4))
                if d == 0:
                    emit_softmax(r, rt)
                ev = evict_p.tile([P, S_CORE], BF16, tag="ev", name=f"ev_{r}_{d}")
                for c in range(NC2):
                    nc.tensor.matmul(
                        pss[c][:], lhsT=bbf[:, d * P:(d + 1) * P],
                        rhs=hid[:, c * NCH:(c + 1) * NCH],
                        start=False, stop=True,
                    )
                    if c % 2 == 0:
                        nc.scalar.activation(
                            out=ev[:, c * NCH:(c + 1) * NCH], in_=pss[c][:],
                            func=mybir.ActivationFunctionType.Copy,
                        )
                    else:
                        nc.vector.tensor_copy(
                            out=ev[:, c * NCH:(c + 1) * NCH], in_=pss[c][:]
                        )
                nc.scalar.dma_start(out=out_r[:, d, :], in_=ev[:])
                if pend is not None:
                    emit_xmo_reduce(nxt, pend)

        emit_consts(0)
        for kt in range(0, KT, 2):
            tiles = emit_xot_piece(0, [kt, kt + 1])
            emit_xmo_reduce(0, tiles)
        for r in range(reps):
            emit_head(r)
            emit_main(r)
            del st[r]
    _split_multi_waits(nc)
    return nc


def prep_inputs(x, W, b, shared_A, shared_B, task_A, task_B, task_emb, collab_weight):
    """Host-side sharding/layout prep: slice/transpose/concat + bf16 cast."""
    x = np.asarray(x, dtype=np.float32)
    W = np.asarray(W, dtype=np.float32)
    b = np.asarray(b, dtype=np.float32)
    a_cat = np.concatenate(
        [np.asarray(shared_A), np.asarray(task_A).reshape(T * R, DIN)], axis=0
    ).astype(np.float32)                                   # [72, DIN]
    a72 = np.ascontiguousarray(
        a_cat.T.reshape(KT, P, A72).transpose(1, 0, 2)
    ).astype(BF16NP)                                       # [P, KT, 72]
    embt = np.ascontiguousarray(
        np.asarray(task_emb, dtype=np.float32).T.reshape(KT, P, T).transpose(1, 0, 2)
    )                                                      # [P, KT, 8] f32
    cwv = np.asarray(collab_weight, dtype=np.float32).reshape(1, 1)

    # full W, d-tiled: [ND, P, KT*P], element [d,p,kt*P+c] = W[d*P+c, kt*P+p]
    wtile = (
        W.T.reshape(KT, P, ND, P)                          # [kt, p, d, c]
        .transpose(2, 1, 0, 3)                             # [d, p, kt, c]
        .reshape(ND, P, KT * P)
    )
    wtile = np.ascontiguousarray(wtile).astype(BF16NP)

    bcat = np.empty((HID, DOUT), dtype=np.float32)
    bcat[0:8] = np.asarray(shared_B).T
    bcat[8:72] = np.asarray(task_B).transpose(0, 2, 1).reshape(T * R, DOUT)
    bcat[72] = b
    bcat = bcat.astype(BF16NP)

    xt = [np.ascontiguousarray(x[p].T).astype(BF16NP) for p in range(B)]  # [DIN, S]

    in_maps = []
    for i in range(N_CORES):
        p, h = i // 2, i % 2
        in_maps.append({
            "xmy": np.ascontiguousarray(xt[p][:, h * S_CORE:(h + 1) * S_CORE]),
            "xot": np.ascontiguousarray(xt[p][:, (1 - h) * S_CORE:(2 - h) * S_CORE]),
            "wt": wtile, "a72": a72, "emb": embt, "bcat": bcat, "cw": cwv,
        })
    return in_maps


def assemble(results):
    out = np.empty((B, S, DOUT), dtype=np.float32)
    for i in range(N_CORES):
        p, h = i // 2, i % 2
        out[p, h * S_CORE:(h + 1) * S_CORE, :] = (
            results[i]["out"].astype(np.float32).T
        )
    return out


_NC_CACHE = None


def kernel(**inputs) -> np.ndarray:
    global _NC_CACHE
    if _NC_CACHE is None:
        _NC_CACHE = build_nc()
    in_maps = prep_inputs(**inputs)
    res = run_bass_kernel_spmd(_NC_CACHE, in_maps, core_ids=list(range(N_CORES)))
    return assemble(res.results)
